# revision 1
# baseline (speedup 1.0000x reference)
"""BWGNN (Beta Wavelet GNN) Trainium2 kernel, 8-way SPMD.

Math (reference.py): deg = out-degree(src) clamped >=1; Dinv = deg^-1/2;
h = leaky_relu(feature @ W + b); L feat = feat - Dinv*segsum_dst(Dinv[src]*feat[src]);
out = concat_i sum_k THETA[i][k] L^k h.

We iterate on u_k = Dinv * L^k h:
    u_{k+1} = u_k - Dinv^2 * segsum_dst(u_k[src])
    out_i   = (sum_k THETA[i][k] u_k) * deg^{1/2}

Distribution: nodes dst-sharded over 8 cores (12500 + pad -> 12544 rows/core).
Full u-table [8*12544, 64] lives in each core's HBM, refreshed per hop by an
AllGather of the per-core updated slices (same-chip AG is cheap).

Per hop per core: edges owned by the core (dst in slice) are bucketed by
(dst-window of 128 nodes, src-chunk of 25088 padded rows), each bucket padded
to a multiple of 128 edges. dma_gather (SWDGE, int16 local idx) pulls
u[src] rows from the HBM table into SBUF edge-tiles; a one-hot matrix
S[p, j] = (dst_loc[p] == j) built on DVE turns the per-window segment-sum
into PE matmuls accumulating in PSUM; the window's PSUM drains through the
u-update (2 DVE ops) into the next-u SBUF slab.
"""

import math
import os
import sys

sys.path.insert(0, "/opt/trn_rl_repo")

import numpy as np

# ---------------------------------------------------------------- constants
N = 100000
E = 1600000
F_IN = 128
H = 64
NCORES = 8
NPC = 12500          # nodes per core
WPC = 98             # windows (128-node groups) per core
NPC_PAD = WPC * 128  # 12544
NCHUNK = 4
CH_NODES = 25000     # original nodes per chunk
CH_PAD = 2 * NPC_PAD  # 25088 padded rows per chunk
NTAB = NCORES * NPC_PAD  # 100352
G_WIN = 7            # windows per phase
NPHASE = WPC // G_WIN  # 14
SBATCH = 8           # S matrices built per DVE instruction
NHOP = 3


def _calculate_theta2(d):
    thetas = []
    for i in range(d):
        c1 = np.zeros(i + 1)
        c1[i] = 0.5 ** i
        c2 = np.array([math.comb(d - i, j) * (-0.5) ** j for j in range(d - i + 1)])
        c = np.convolve(c1, c2)
        B = math.factorial(i) * math.factorial(d - i) / math.factorial(d + 1)
        c = c / (2.0 * B)
        thetas.append([float(c[j]) for j in range(d)])
    return thetas


THETAS = _calculate_theta2(4)  # [4][4], theta[i][k] weight of L^k h in output i


# ---------------------------------------------------------------- host prep
def _prep(edge_index: np.ndarray):
    """Bucket edges, build per-core gather-index / dst-loc arrays and the
    static tile-count table T[w][k] (shared by all cores)."""
    src = edge_index[0].astype(np.int64)
    dst = edge_index[1].astype(np.int64)

    deg = np.bincount(src, minlength=N).astype(np.float32)
    dinv = np.maximum(deg, np.float32(1.0)) ** np.float32(-0.5)
    dinv2 = dinv * dinv
    dsqrt = np.float32(1.0) / dinv  # = max(deg,1)^0.5

    core = dst // NPC
    w = (dst % NPC) // 128
    dst_loc = (dst % NPC) % 128
    k = src // CH_NODES
    # padded row within chunk
    src_loc = (src % NPC) + (src // NPC - 2 * k) * NPC_PAD

    bucket = ((core * WPC + w) * NCHUNK + k)
    cnt = np.bincount(bucket, minlength=NCORES * WPC * NCHUNK).reshape(
        NCORES, WPC, NCHUNK
    )
    T = np.maximum(1, -(-cnt // 128)).max(axis=0)  # [WPC, NCHUNK] int64
    ncols = T * 1  # tiles per (w,k)

    # sort edges by bucket (stable, any order within bucket)
    order = np.argsort(bucket, kind="stable")
    src_loc_s = src_loc[order]
    dst_loc_s = dst_loc[order]
    bucket_s = bucket[order]
    # start offset of each (c,w,k) in the sorted arrays
    starts = np.zeros(NCORES * WPC * NCHUNK + 1, dtype=np.int64)
    np.cumsum(np.bincount(bucket_s, minlength=NCORES * WPC * NCHUNK), out=starts[1:])

    # global slot enumeration per core: for p, k, w in phase, t, slot
    # (tile g covers slots [128g, 128(g+1)))
    tot_tiles = int(T.sum())
    tot_slots = tot_tiles * 128

    # per-(p,k): column base within phase msgs tile and call length
    # call order within phase: k = 0..3
    phase_tiles = []  # [p] -> total tiles in phase
    call_info = []  # [p][k] = (idx_off_slots, n_slots, col_base)
    gcol = 0  # running global tile index
    for p in range(NPHASE):
        ws = range(p * G_WIN, (p + 1) * G_WIN)
        info = []
        col = 0
        for kk in range(NCHUNK):
            n_t = int(sum(T[ww][kk] for ww in ws))
            info.append((gcol * 128, n_t * 128, col))
            col += n_t
            gcol += n_t
        call_info.append(info)
        phase_tiles.append(col)
    assert gcol == tot_tiles

    # map (w,k) -> global tile start
    tile_start = np.zeros((WPC, NCHUNK), dtype=np.int64)
    g = 0
    for p in range(NPHASE):
        for kk in range(NCHUNK):
            for ww in range(p * G_WIN, (p + 1) * G_WIN):
                tile_start[ww][kk] = g
                g += T[ww][kk]

    # build per-core slot arrays
    per_core = []
    for c in range(NCORES):
        idx_arr = np.zeros(tot_slots, dtype=np.int16)
        dl_arr = np.full(tot_slots, -1.0, dtype=np.float32)
        for ww in range(WPC):
            for kk in range(NCHUNK):
                b = (c * WPC + ww) * NCHUNK + kk
                s0, s1 = starts[b], starts[b + 1]
                n = s1 - s0
                o = tile_start[ww][kk] * 128
                cap = T[ww][kk] * 128
                assert n <= cap
                idx_arr[o : o + n] = src_loc_s[s0:s1]
                dl_arr[o : o + n] = dst_loc_s[s0:s1]
                if n < cap:  # pad with a valid idx (row 0 of chunk), dst -1
                    idx_arr[o + n : o + cap] = 0
        # wrap idx: position i -> [16r + i%16, i//16]
        idx_w = idx_arr.reshape(-1, 16).T  # [16, tot/16]
        idx_w = np.tile(idx_w, (8, 1))  # [128, tot/16]
        # dst_loc: tile g slot s -> [s, g]
        dl_w = dl_arr.reshape(tot_tiles, 128).T.copy()  # [128, tot_tiles]
        # pad dstloc columns for S-batch overrun
        dl_w = np.concatenate(
            [dl_w, np.full((128, SBATCH), -1.0, dtype=np.float32)], axis=1
        )
        per_core.append((idx_w, dl_w))

    # per-core dinv arrays [128, WPC] (pad nodes -> 1.0)
    def slice_arr(a):
        out = np.ones((NCORES, NPC_PAD), dtype=np.float32)
        out[:, :NPC] = a.reshape(NCORES, NPC)
        return out.reshape(NCORES, WPC, 128).transpose(0, 2, 1).copy()

    return {
        "T": T,
        "tot_tiles": tot_tiles,
        "phase_tiles": phase_tiles,
        "call_info": call_info,
        "tile_start": tile_start,
        "per_core": per_core,
        "dinv_t": slice_arr(dinv),
        "dinv2_t": slice_arr(dinv2),
        "dsqrt_t": slice_arr(dsqrt),
    }


# ---------------------------------------------------------------- bass build
def _build_nc(T, tot_tiles, phase_tiles, call_info, tile_start, reps=1):
    import concourse.bacc as bacc
    import concourse.mybir as mybir
    import concourse.tile as tile
    from concourse.library_config import mlp

    f32 = mybir.dt.float32
    i16 = mybir.dt.int16
    Alu = mybir.AluOpType

    tot16 = tot_tiles * 128 // 16
    dl_cols = tot_tiles + SBATCH
    max_ptiles = max(phase_tiles)
    nsb = -(-tot_tiles // SBATCH)  # number of S-batches overall

    nc = bacc.Bacc("TRN2", target_bir_lowering=False, debug=False,
                   num_devices=NCORES)

    feat_in = nc.dram_tensor("feat_in", [NPC_PAD, F_IN], f32, kind="ExternalInput")
    w_in = nc.dram_tensor("w_in", [F_IN, H], f32, kind="ExternalInput")
    b_in = nc.dram_tensor("b_in", [1, H], f32, kind="ExternalInput")
    ident_in = nc.dram_tensor("ident_in", [128, 128], f32, kind="ExternalInput")
    iota_in = nc.dram_tensor("iota_in", [128, SBATCH * 128], f32, kind="ExternalInput")
    idx_in = nc.dram_tensor("idx_in", [128, tot16], i16, kind="ExternalInput")
    dstloc_in = nc.dram_tensor("dstloc_in", [128, dl_cols], f32, kind="ExternalInput")
    dinv_in = nc.dram_tensor("dinv_in", [128, WPC], f32, kind="ExternalInput")
    dinv2_in = nc.dram_tensor("dinv2_in", [128, WPC], f32, kind="ExternalInput")
    dsqrt_in = nc.dram_tensor("dsqrt_in", [128, WPC], f32, kind="ExternalInput")
    out = nc.dram_tensor("out", [NPC_PAD, 4 * H], f32, kind="ExternalOutput")

    SLAB = WPC * H  # 6272 free elems

    with tile.TileContext(nc) as tc:
        with (
            tc.tile_pool(name="dram", bufs=1, space="DRAM") as dram,
            tc.tile_pool(name="const", bufs=1) as const,
            tc.tile_pool(name="slabs", bufs=1) as slabs,
            tc.tile_pool(name="work", bufs=3) as work,
            tc.tile_pool(name="msgs_p", bufs=2) as msgs_pool,
            tc.tile_pool(name="psum", bufs=4, space="PSUM") as psum_pool,
        ):
            nc.gpsimd.load_library(mlp)

            # ---------- constants / metadata loads
            w_sb = const.tile([F_IN, H], f32)
            nc.sync.dma_start(out=w_sb[:], in_=w_in[:])
            b_sb = const.tile([1, H], f32)
            nc.sync.dma_start(out=b_sb[:], in_=b_in[:])
            ident = const.tile([128, 128], f32)
            nc.sync.dma_start(out=ident[:], in_=ident_in[:])
            iota_sb = const.tile([128, SBATCH * 128], f32)
            nc.sync.dma_start(out=iota_sb[:], in_=iota_in[:])
            dinv_sb = const.tile([128, WPC], f32)
            nc.sync.dma_start(out=dinv_sb[:], in_=dinv_in[:])
            dinv2_sb = const.tile([128, WPC], f32)
            nc.sync.dma_start(out=dinv2_sb[:], in_=dinv2_in[:])
            dsqrt_sb = const.tile([128, WPC], f32)
            nc.sync.dma_start(out=dsqrt_sb[:], in_=dsqrt_in[:])
            ones_col = const.tile([1, 128], f32)
            nc.vector.memset(ones_col[:], 1.0)

            slab_a = slabs.tile([128, SLAB], f32)  # u0 -> u2
            slab_b = slabs.tile([128, SLAB], f32)  # u1 -> u3

            saves = [
                dram.tile([128, SLAB], f32, name=f"save{kk}") for kk in range(2)
            ]
            ag_in = dram.tile([NPC_PAD, H], f32)

            for rep in range(reps):
                _emit_body(
                    nc, tc, mybir, rep, T, tot_tiles, phase_tiles, call_info,
                    tile_start, dram, work, msgs_pool, psum_pool, slab_a, slab_b,
                    saves, ag_in, feat_in, out, w_sb, b_sb, ident, iota_sb,
                    idx_in, dstloc_in, dinv_sb, dinv2_sb, dsqrt_sb, ones_col,
                )

    nc.compile()
    return nc


def _emit_body(
    nc, tc, mybir, rep, T, tot_tiles, phase_tiles, call_info, tile_start,
    dram, work, msgs_pool, psum_pool, slab_a, slab_b, saves, ag_in, feat_in,
    out, w_sb, b_sb, ident, iota_sb, idx_in, dstloc_in, dinv_sb, dinv2_sb,
    dsqrt_sb, ones_col,
):
    import concourse.mybir as mybir

    f32 = mybir.dt.float32
    i16 = mybir.dt.int16
    Alu = mybir.AluOpType
    SLAB = WPC * H
    max_ptiles = max(phase_tiles)

    SKIP_COMPUTE = bool(int(os.environ.get("BW_SKIP_COMPUTE", "0")))
    SKIP_GATHER = bool(int(os.environ.get("BW_SKIP_GATHER", "0")))
    if True:  # keep indentation of original body
            # ---------- u0 = Dinv * leaky_relu(feat @ W + b)
            with tc.tile_pool(name=f"featp{rep}", bufs=2) as featp:
                for w0 in range(0, WPC, G_WIN):
                  gw = min(G_WIN, WPC - w0)
                  feat_sb = featp.tile([128, G_WIN, F_IN], f32, tag="feat_sb")
                  nc.sync.dma_start(
                      out=feat_sb[:, :gw, :],
                      in_=feat_in[w0 * 128 : (w0 + gw) * 128, :].rearrange(
                          "(w p) f -> p w f", p=128
                      ),
                  )
                  for w in range(w0, w0 + gw):
                    ftT_ps = psum_pool.tile([128, 128], f32, tag="ps", bufs=8)
                    nc.tensor.transpose(
                        out=ftT_ps[:], in_=feat_sb[:, w - w0, :], identity=ident[:]
                    )
                    ftT = work.tile([128, 128], f32, tag="ftT_sb")
                    nc.vector.tensor_copy(out=ftT[:], in_=ftT_ps[:])
                    h_ps = psum_pool.tile([128, 128], f32, tag="ps", bufs=8)
                    nc.tensor.matmul(
                        out=h_ps[:, :H], lhsT=ftT[:], rhs=w_sb[:],
                        start=True, stop=False,
                    )
                    nc.tensor.matmul(
                        out=h_ps[:, :H], lhsT=ones_col[:], rhs=b_sb[:],
                        start=False, stop=True,
                    )
                    t1 = work.tile([128, H], f32, tag="t1")
                    nc.vector.tensor_scalar(
                        out=t1[:], in0=h_ps[:, :H], scalar1=0.01, scalar2=None,
                        op0=Alu.mult,
                    )
                    t2 = work.tile([128, H], f32, tag="t2")
                    nc.vector.tensor_tensor(
                        out=t2[:], in0=h_ps[:, :H], in1=t1[:], op=Alu.max
                    )
                    nc.vector.tensor_scalar(
                        out=slab_a[:, w * H : (w + 1) * H], in0=t2[:],
                        scalar1=dinv_sb[:, w : w + 1], scalar2=None, op0=Alu.mult,
                    )

            def store_slab(slab, save_idx):
                if save_idx is not None:
                    nc.sync.dma_start(out=saves[save_idx][:], in_=slab[:])
                nc.sync.dma_start(
                    out=ag_in[:].rearrange("(w p) h -> p w h", p=128),
                    in_=slab[:].rearrange("p (w h) -> p w h", h=H),
                )

            store_slab(slab_a, 0)

            # ---------- hops
            cur, nxt = slab_a, slab_b
            for hop in range(NHOP):
                table = dram.tile(
                    [NTAB, H], f32, addr_space="Shared", name=f"table{rep}_{hop}"
                )
                nc.gpsimd.collective_compute(
                    "AllGather",
                    Alu.bypass,
                    replica_groups=[list(range(NCORES))],
                    ins=[ag_in.opt()],
                    outs=[table.opt()],
                )

                g = 0  # global tile counter
                for p in range(NPHASE):
                    ptiles = phase_tiles[p]
                    p_off16 = call_info[p][0][0] // 16  # phase idx col start
                    p_len16 = ptiles * 128 // 16
                    first_g_p = call_info[p][0][0] // 128
                    idxp = msgs_pool.tile(
                        [128, (max_ptiles * 128) // 16], i16, tag="idxp"
                    )
                    nc.sync.dma_start(
                        out=idxp[:, :p_len16],
                        in_=idx_in[:, p_off16 : p_off16 + p_len16],
                    )
                    dstp = msgs_pool.tile(
                        [128, max_ptiles + SBATCH], f32, tag="dstp"
                    )
                    nc.sync.dma_start(
                        out=dstp[:, : ptiles + SBATCH],
                        in_=dstloc_in[:, first_g_p : first_g_p + ptiles + SBATCH],
                    )
                    msgs = msgs_pool.tile([128, max_ptiles, H], f32, tag="msgs")
                    if SKIP_GATHER and (hop > 0 or p > 1):
                        nc.vector.memset(msgs[:, 0, :], 0.5)
                    GCAP = 8192  # max idxs per dma_gather (desc-ring capacity)
                    for kk in range(NCHUNK):
                        off_sl, n_sl, col = call_info[p][kk]
                        if SKIP_GATHER and (hop > 0 or p > 1):
                            continue
                        for o in range(0, n_sl, GCAP):
                            ln = min(GCAP, n_sl - o)
                            c0 = col + o // 128
                            i0 = (off_sl + o) // 16 - p_off16
                            nc.gpsimd.dma_gather(
                                msgs[:, c0 : c0 + ln // 128, :],
                                table[CH_PAD * kk : CH_PAD * (kk + 1), :],
                                idxp[:, i0 : i0 + ln // 16],
                                ln,
                                ln,
                                H,
                                single_packet=False,
                            )
                    # S build for this phase's tiles
                    first_g = g
                    s_tiles = []
                    for sb0 in range(0, ptiles, SBATCH):
                        S_big = work.tile(
                            [128, SBATCH * 128], f32, tag="S", bufs=4
                        )
                        if SKIP_COMPUTE:
                            s_tiles.append(S_big)
                            continue
                        nc.vector.tensor_tensor(
                            out=S_big[:].rearrange("p (t j) -> p t j", j=128),
                            in0=iota_sb[:].rearrange("p (t j) -> p t j", j=128),
                            in1=dstp[:, sb0 : sb0 + SBATCH]
                            .to_broadcast([128, SBATCH, 128]),
                            op=Alu.is_equal,
                        )
                        s_tiles.append(S_big)
                    # matmuls per window
                    for ww in range(p * G_WIN, (p + 1) * G_WIN):
                        agg_ps = psum_pool.tile([128, 128], f32, tag="ps", bufs=8)
                        if SKIP_COMPUTE:
                            nc.vector.memset(agg_ps[:, :H], 0.0)
                        n_mm = int(sum(T[ww][kk] for kk in range(NCHUNK)))
                        mm_i = 0
                        for kk in range(NCHUNK if not SKIP_COMPUTE else 0):
                            _, _, col = call_info[p][kk]
                            # tiles of (ww,kk) within the call: windows before ww
                            cbase = col + int(
                                sum(T[w2][kk] for w2 in range(p * G_WIN, ww))
                            )
                            for t in range(int(T[ww][kk])):
                                # global tile index in host (p,k,w,t) order
                                lg = int(tile_start[ww][kk]) + t - first_g
                                S_t = s_tiles[lg // SBATCH]
                                j0 = (lg % SBATCH) * 128
                                nc.tensor.matmul(
                                    out=agg_ps[:, :H],
                                    lhsT=S_t[:, j0 : j0 + 128],
                                    rhs=msgs[:, cbase + t, :],
                                    start=(mm_i == 0),
                                    stop=(mm_i == n_mm - 1),
                                )
                                mm_i += 1
                        g += n_mm
                        # u' = u - dinv2 * agg
                        tscl = work.tile([128, H], f32, tag="tscl")
                        nc.vector.tensor_scalar(
                            out=tscl[:], in0=agg_ps[:, :H],
                            scalar1=dinv2_sb[:, ww : ww + 1], scalar2=None,
                            op0=Alu.mult,
                        )
                        nc.vector.tensor_tensor(
                            out=nxt[:, ww * H : (ww + 1) * H],
                            in0=cur[:, ww * H : (ww + 1) * H],
                            in1=tscl[:],
                            op=Alu.subtract,
                        )
                assert g == sum(phase_tiles[:NPHASE])
                if hop == 0:
                    store_slab(nxt, 1)  # save u1
                elif hop < NHOP - 1:
                    store_slab(nxt, None)  # u2 stays in SBUF (slab_a)
                cur, nxt = nxt, cur

            # after loop: slab_a = u2, slab_b = u3 (cur=u3, nxt=u2)
            u3, u2 = cur, nxt

            # ---------- combine: out_i = (sum_k theta_ik u_k) * dsqrt
            # chunked over window groups to bound SBUF
            CGRP = 7  # windows per combine group
            with tc.tile_pool(name=f"comb{rep}", bufs=2) as comb:
                for w0 in range(0, WPC, CGRP):
                    nw = min(CGRP, WPC - w0)
                    cs = slice(w0 * H, (w0 + nw) * H)
                    u0c = comb.tile([128, CGRP * H], f32, tag="u0c")
                    nc.sync.dma_start(out=u0c[:, : nw * H], in_=saves[0][:, cs])
                    u1c = comb.tile([128, CGRP * H], f32, tag="u1c")
                    nc.sync.dma_start(out=u1c[:, : nw * H], in_=saves[1][:, cs])
                    us = [u0c[:, : nw * H], u1c[:, : nw * H], u2[:, cs], u3[:, cs]]
                    for i in range(4):
                        acc = comb.tile([128, CGRP * H], f32, tag="acc")
                        a = acc[:, : nw * H]
                        ks = [kk for kk in range(4) if THETAS[i][kk] != 0.0]
                        nc.vector.tensor_scalar(
                            out=a, in0=us[ks[0]],
                            scalar1=float(THETAS[i][ks[0]]), scalar2=None,
                            op0=Alu.mult,
                        )
                        for kk in ks[1:]:
                            tmp = comb.tile([128, CGRP * H], f32, tag="ctmp")
                            nc.vector.tensor_scalar(
                                out=tmp[:, : nw * H], in0=us[kk],
                                scalar1=float(THETAS[i][kk]), scalar2=None,
                                op0=Alu.mult,
                            )
                            nc.vector.tensor_tensor(
                                out=a, in0=a, in1=tmp[:, : nw * H], op=Alu.add
                            )
                        nc.vector.tensor_tensor(
                            out=a.rearrange("p (w h) -> p w h", h=H),
                            in0=a.rearrange("p (w h) -> p w h", h=H),
                            in1=dsqrt_sb[:, w0 : w0 + nw].to_broadcast(
                                [128, nw, H]
                            ),
                            op=Alu.mult,
                        )
                        nc.sync.dma_start(
                            out=out[
                                w0 * 128 : (w0 + nw) * 128, i * H : (i + 1) * H
                            ].rearrange("(w p) h -> p w h", p=128),
                            in_=a.rearrange("p (w h) -> p w h", h=H),
                        )


# ---------------------------------------------------------------- runner
def _make_runner(nc, in_maps, n_cores):
    import jax
    import numpy as np
    from jax.sharding import Mesh, NamedSharding, PartitionSpec
    from jax.experimental.shard_map import shard_map

    import concourse.mybir as mybir
    from concourse import bass2jax

    bass2jax.install_neuronx_cc_hook()
    partition_name = nc.partition_id_tensor.name if nc.partition_id_tensor else None
    in_names, out_names, out_avals, zero_outs = [], [], [], []
    for alloc in nc.m.functions[0].allocations:
        if not isinstance(alloc, mybir.MemoryLocationSet):
            continue
        name = alloc.memorylocations[0].name
        if alloc.kind == "ExternalInput":
            if name != partition_name:
                in_names.append(name)
        elif alloc.kind == "ExternalOutput":
            out_names.append(name)
            shape = tuple(alloc.tensor_shape)
            dtype = mybir.dt.np(alloc.dtype)
            out_avals.append(jax.core.ShapedArray(shape, dtype))
            zero_outs.append(np.zeros(shape, dtype))
    n_params = len(in_names)
    all_in_names = list(in_names) + list(out_names)
    if partition_name is not None:
        all_in_names.append(partition_name)

    def _body(*args):
        operands = list(args)
        if partition_name is not None:
            operands.append(bass2jax.partition_id_tensor())
        outs = bass2jax._bass_exec_p.bind(
            *operands,
            out_avals=tuple(out_avals),
            in_names=tuple(all_in_names),
            out_names=tuple(out_names),
            lowering_input_output_aliases=(),
            sim_require_finite=True,
            sim_require_nnan=True,
            nc=nc,
        )
        return tuple(outs)

    devices = jax.devices()[:n_cores]
    mesh = Mesh(np.asarray(devices), ("core",))
    n_ops = n_params + len(out_names)
    sharded = jax.jit(
        shard_map(
            _body,
            mesh=mesh,
            in_specs=(PartitionSpec("core"),) * n_ops,
            out_specs=(PartitionSpec("core"),) * len(out_names),
            check_rep=False,
        ),
        keep_unused=True,
    )
    sh = NamedSharding(mesh, PartitionSpec("core"))
    concat_in = [
        jax.device_put(
            np.concatenate([np.asarray(in_maps[c][nm]) for c in range(n_cores)], 0),
            sh,
        )
        for nm in in_names
    ]
    concat_zeros = [
        jax.device_put(np.zeros((n_cores * z.shape[0], *z.shape[1:]), z.dtype), sh)
        for z in zero_outs
    ]
    args = concat_in + concat_zeros

    def run():
        return sharded(*args)

    return run, out_names, out_avals


_CACHE = {}


def _get_built(edge_index_bytes_key, edge_index):
    if edge_index_bytes_key not in _CACHE:
        prep = _prep(edge_index)
        nc = _build_nc(
            prep["T"],
            prep["tot_tiles"],
            prep["phase_tiles"],
            prep["call_info"],
            prep["tile_start"],
        )
        _CACHE[edge_index_bytes_key] = (prep, nc)
    return _CACHE[edge_index_bytes_key]


def _make_in_maps(prep, inputs):
    feature = np.asarray(inputs["feature"], dtype=np.float32)
    W = np.asarray(inputs["W"], dtype=np.float32)
    b = np.asarray(inputs["b"], dtype=np.float32)

    iota = np.broadcast_to(
        np.tile(np.arange(128, dtype=np.float32), SBATCH), (128, SBATCH * 128)
    ).copy()
    ident = np.eye(128, dtype=np.float32)
    b2 = b.reshape(1, H)

    feat_pad = np.zeros((NCORES, NPC_PAD, F_IN), dtype=np.float32)
    feat_pad[:, :NPC, :] = feature.reshape(NCORES, NPC, F_IN)

    in_maps = []
    for c in range(NCORES):
        idx_w, dl_w = prep["per_core"][c]
        in_maps.append(
            {
                "feat_in": feat_pad[c],
                "w_in": W,
                "b_in": b2,
                "ident_in": ident,
                "iota_in": iota,
                "idx_in": idx_w,
                "dstloc_in": dl_w,
                "dinv_in": prep["dinv_t"][c],
                "dinv2_in": prep["dinv2_t"][c],
                "dsqrt_in": prep["dsqrt_t"][c],
            }
        )
    return in_maps


def kernel(feature, edge_index, W, b):
    import jax

    edge_index = np.asarray(edge_index, dtype=np.int32)
    key = hash(edge_index.tobytes())
    prep, nc = _get_built(key, edge_index)
    in_maps = _make_in_maps(prep, {"feature": feature, "W": W, "b": b})

    run, out_names, out_avals = _make_runner(nc, in_maps, NCORES)
    outs = jax.block_until_ready(run())
    oi = out_names.index("out")
    full = np.asarray(outs[oi]).reshape(NCORES, NPC_PAD, 4 * H)
    return full[:, :NPC, :].reshape(N, 4 * H)


if __name__ == "__main__":
    rng = np.random.default_rng(0)
    feature = rng.standard_normal((N, F_IN), dtype=np.float32)
    edge_index = rng.integers(0, N, (2, E)).astype(np.int32)
    W = (rng.standard_normal((F_IN, H)) * 0.05).astype(np.float32)
    b = (rng.standard_normal((H,)) * 0.05).astype(np.float32)
    out = kernel(feature=feature, edge_index=edge_index, W=W, b=b)
    print(out.shape, out.dtype, float(np.abs(out).mean()))



# revision 19
# speedup vs baseline: 1.4182x; 1.4182x over previous
"""BWGNN (Beta Wavelet GNN) Trainium2 kernel, 8-way SPMD.

Math (reference.py): deg = out-degree(src) clamped >=1; Dinv = deg^-1/2;
h = leaky_relu(feature @ W + b); L feat = feat - Dinv*segsum_dst(Dinv[src]*feat[src]);
out = concat_i sum_k THETA[i][k] L^k h.

We iterate on u_k = Dinv * L^k h:
    u_{k+1} = u_k - Dinv^2 * segsum_dst(u_k[src])
    out_i   = (sum_k THETA[i][k] u_k) * deg^{1/2}

Distribution: nodes dst-sharded over 8 cores (12500 + pad -> 12544 rows/core).
Full u-table [8*12544, 64] lives in each core's HBM, refreshed per hop by an
AllGather of the per-core updated slices (same-chip AG is cheap).

Per hop per core: edges owned by the core (dst in slice) are bucketed by
(dst-window of 128 nodes, src-chunk of 25088 padded rows), each bucket padded
to a multiple of 128 edges. dma_gather (SWDGE, int16 local idx) pulls
u[src] rows from the HBM table into SBUF edge-tiles; a one-hot matrix
S[p, j] = (dst_loc[p] == j) built on DVE turns the per-window segment-sum
into PE matmuls accumulating in PSUM; the window's PSUM drains through the
u-update (2 DVE ops) into the next-u SBUF slab.
"""

import math
import os
import sys

sys.path.insert(0, "/opt/trn_rl_repo")

import numpy as np

# ---------------------------------------------------------------- constants
N = 100000
E = 1600000
F_IN = 128
H = 64
NCORES = 8
NPC = 12500          # nodes per core
WPC = 98             # windows (128-node groups) per core
NPC_PAD = WPC * 128  # 12544
NCHUNK = 4
CH_NODES = 25000     # original nodes per chunk
CH_PAD = 2 * NPC_PAD  # 25088 padded rows per chunk
NTAB = NCORES * NPC_PAD  # 100352
G_WIN = 7            # windows per phase
NPHASE = WPC // G_WIN  # 14
SBATCH = 8           # S matrices built per DVE instruction
NHOP = 3


def _calculate_theta2(d):
    thetas = []
    for i in range(d):
        c1 = np.zeros(i + 1)
        c1[i] = 0.5 ** i
        c2 = np.array([math.comb(d - i, j) * (-0.5) ** j for j in range(d - i + 1)])
        c = np.convolve(c1, c2)
        B = math.factorial(i) * math.factorial(d - i) / math.factorial(d + 1)
        c = c / (2.0 * B)
        thetas.append([float(c[j]) for j in range(d)])
    return thetas


THETAS = _calculate_theta2(4)  # [4][4], theta[i][k] weight of L^k h in output i


# ---------------------------------------------------------------- host prep
def _prep(edge_index: np.ndarray):
    """Bucket edges, build per-core gather-index / dst-loc arrays and the
    static tile-count table T[w][k] (shared by all cores)."""
    src = edge_index[0].astype(np.int64)
    dst = edge_index[1].astype(np.int64)

    deg = np.bincount(src, minlength=N).astype(np.float32)
    dinv = np.maximum(deg, np.float32(1.0)) ** np.float32(-0.5)
    dinv2 = dinv * dinv
    dsqrt = np.float32(1.0) / dinv  # = max(deg,1)^0.5

    core = dst // NPC
    w = (dst % NPC) // 128
    dst_loc = (dst % NPC) % 128
    k = src // CH_NODES
    # padded row within chunk
    src_loc = (src % NPC) + (src // NPC - 2 * k) * NPC_PAD

    bucket = ((core * WPC + w) * NCHUNK + k)
    cnt = np.bincount(bucket, minlength=NCORES * WPC * NCHUNK).reshape(
        NCORES, WPC, NCHUNK
    )
    T = np.maximum(1, -(-cnt // 128)).max(axis=0)  # [WPC, NCHUNK] int64
    ncols = T * 1  # tiles per (w,k)

    # sort edges by bucket (stable, any order within bucket).  In
    # BW_IDX_MODE=sort, additionally order each bucket's edges by source row
    # so consecutive gather descriptors hit nearby HBM addresses (segment-sum
    # is order-invariant, so results are unchanged).
    idx_mode = os.environ.get("BW_IDX_MODE", "")
    if idx_mode == "sort":
        order = np.lexsort((src_loc, bucket))
    else:
        order = np.argsort(bucket, kind="stable")
    src_loc_s = src_loc[order]
    dst_loc_s = dst_loc[order]
    bucket_s = bucket[order]
    # start offset of each (c,w,k) in the sorted arrays
    starts = np.zeros(NCORES * WPC * NCHUNK + 1, dtype=np.int64)
    np.cumsum(np.bincount(bucket_s, minlength=NCORES * WPC * NCHUNK), out=starts[1:])

    # global slot enumeration per core: for p, k, w in phase, t, slot
    # (tile g covers slots [128g, 128(g+1)))
    tot_tiles = int(T.sum())
    tot_slots = tot_tiles * 128

    # per-(p,k): column base within phase msgs tile and call length
    # call order within phase: k = 0..3
    phase_tiles = []  # [p] -> total tiles in phase
    call_info = []  # [p][k] = (idx_off_slots, n_slots, col_base)
    gcol = 0  # running global tile index
    for p in range(NPHASE):
        ws = range(p * G_WIN, (p + 1) * G_WIN)
        info = []
        col = 0
        for kk in range(NCHUNK):
            n_t = int(sum(T[ww][kk] for ww in ws))
            info.append((gcol * 128, n_t * 128, col))
            col += n_t
            gcol += n_t
        call_info.append(info)
        phase_tiles.append(col)
    assert gcol == tot_tiles

    # map (w,k) -> global tile start
    tile_start = np.zeros((WPC, NCHUNK), dtype=np.int64)
    g = 0
    for p in range(NPHASE):
        for kk in range(NCHUNK):
            for ww in range(p * G_WIN, (p + 1) * G_WIN):
                tile_start[ww][kk] = g
                g += T[ww][kk]

    # build per-core slot arrays
    per_core = []
    for c in range(NCORES):
        idx_arr = np.zeros(tot_slots, dtype=np.int16)
        dl_arr = np.full(tot_slots, -1.0, dtype=np.float32)
        for ww in range(WPC):
            for kk in range(NCHUNK):
                b = (c * WPC + ww) * NCHUNK + kk
                s0, s1 = starts[b], starts[b + 1]
                n = s1 - s0
                o = tile_start[ww][kk] * 128
                cap = T[ww][kk] * 128
                assert n <= cap
                idx_arr[o : o + n] = src_loc_s[s0:s1]
                dl_arr[o : o + n] = dst_loc_s[s0:s1]
                if n < cap:  # pad with a valid idx (row 0 of chunk), dst -1
                    idx_arr[o + n : o + cap] = 0
        if idx_mode == "zero":  # timing probe: every gather hits row 0
            idx_arr[:] = 0
        # wrap idx: position i -> [16r + i%16, i//16]
        idx_w = idx_arr.reshape(-1, 16).T  # [16, tot/16]
        idx_w = np.tile(idx_w, (8, 1))  # [128, tot/16]
        # dst_loc: tile g slot s -> [s, g]
        dl_w = dl_arr.reshape(tot_tiles, 128).T.copy()  # [128, tot_tiles]
        # pad dstloc columns for S-batch overrun
        dl_w = np.concatenate(
            [dl_w, np.full((128, SBATCH), -1.0, dtype=np.float32)], axis=1
        )
        per_core.append((idx_w, dl_w))

    # per-core dinv arrays [128, WPC] (pad nodes -> 1.0)
    def slice_arr(a):
        out = np.ones((NCORES, NPC_PAD), dtype=np.float32)
        out[:, :NPC] = a.reshape(NCORES, NPC)
        return out.reshape(NCORES, WPC, 128).transpose(0, 2, 1).copy()

    return {
        "T": T,
        "tot_tiles": tot_tiles,
        "phase_tiles": phase_tiles,
        "call_info": call_info,
        "tile_start": tile_start,
        "per_core": per_core,
        "dinv_t": slice_arr(dinv),
        "dinv2_t": slice_arr(dinv2),
        "dsqrt_t": slice_arr(dsqrt),
    }


# ---------------------------------------------------------------- bass build
def _build_nc(T, tot_tiles, phase_tiles, call_info, tile_start, reps=1):
    import concourse.bacc as bacc
    import concourse.mybir as mybir
    import concourse.tile as tile
    from concourse.library_config import mlp

    f32 = mybir.dt.float32
    i16 = mybir.dt.int16
    Alu = mybir.AluOpType

    tot16 = tot_tiles * 128 // 16
    dl_cols = tot_tiles + SBATCH
    max_ptiles = max(phase_tiles)
    nsb = -(-tot_tiles // SBATCH)  # number of S-batches overall

    nq = int(os.environ.get("BW_NSWDGE", "4"))
    nc = bacc.Bacc("TRN2", target_bir_lowering=False, debug=False,
                   num_devices=NCORES, num_swdge_queues=nq)

    feat_in = nc.dram_tensor("feat_in", [NPC_PAD, F_IN], f32, kind="ExternalInput")
    w_in = nc.dram_tensor("w_in", [F_IN, H], f32, kind="ExternalInput")
    b_in = nc.dram_tensor("b_in", [1, H], f32, kind="ExternalInput")
    ident_in = nc.dram_tensor("ident_in", [128, 128], f32, kind="ExternalInput")
    iota_in = nc.dram_tensor("iota_in", [128, SBATCH * 128], f32, kind="ExternalInput")
    idx_in = nc.dram_tensor("idx_in", [128, tot16], i16, kind="ExternalInput")
    dstloc_in = nc.dram_tensor("dstloc_in", [128, dl_cols], f32, kind="ExternalInput")
    dinv_in = nc.dram_tensor("dinv_in", [128, WPC], f32, kind="ExternalInput")
    dinv2_in = nc.dram_tensor("dinv2_in", [128, WPC], f32, kind="ExternalInput")
    dsqrt_in = nc.dram_tensor("dsqrt_in", [128, WPC], f32, kind="ExternalInput")
    out = nc.dram_tensor("out", [NPC_PAD, 4 * H], f32, kind="ExternalOutput")

    SLAB = WPC * H  # 6272 free elems

    with tile.TileContext(nc) as tc:
        with (
            tc.tile_pool(name="dram", bufs=1, space="DRAM") as dram,
            tc.tile_pool(name="const", bufs=1) as const,
            tc.tile_pool(name="slabs", bufs=1) as slabs,
            tc.tile_pool(name="work", bufs=3) as work,
            tc.tile_pool(name="msgs_p", bufs=2) as msgs_pool,
            tc.tile_pool(name="psum", bufs=4, space="PSUM") as psum_pool,
        ):
            nc.gpsimd.load_library(mlp)

            # ---------- constants / metadata loads
            w_sb = const.tile([F_IN, H], f32)
            nc.sync.dma_start(out=w_sb[:], in_=w_in[:])
            b_sb = const.tile([1, H], f32)
            nc.sync.dma_start(out=b_sb[:], in_=b_in[:])
            ident = const.tile([128, 128], f32)
            nc.sync.dma_start(out=ident[:], in_=ident_in[:])
            iota_sb = const.tile([128, SBATCH * 128], f32)
            nc.sync.dma_start(out=iota_sb[:], in_=iota_in[:])
            dinv_sb = const.tile([128, WPC], f32)
            nc.sync.dma_start(out=dinv_sb[:], in_=dinv_in[:])
            dinv2_sb = const.tile([128, WPC], f32)
            nc.sync.dma_start(out=dinv2_sb[:], in_=dinv2_in[:])
            dsqrt_sb = const.tile([128, WPC], f32)
            nc.sync.dma_start(out=dsqrt_sb[:], in_=dsqrt_in[:])
            ones_col = const.tile([1, 128], f32)
            nc.vector.memset(ones_col[:], 1.0)

            slab_a = slabs.tile([128, SLAB], f32)  # u0 -> u2
            slab_b = slabs.tile([128, SLAB], f32)  # u1 -> u3

            saves = [
                dram.tile([128, SLAB], f32, name=f"save{kk}") for kk in range(2)
            ]
            ag_in = dram.tile([NPC_PAD, H], f32)

            for rep in range(reps):
                _emit_body(
                    nc, tc, mybir, rep, T, tot_tiles, phase_tiles, call_info,
                    tile_start, dram, work, msgs_pool, psum_pool, slab_a, slab_b,
                    saves, ag_in, feat_in, out, w_sb, b_sb, ident, iota_sb,
                    idx_in, dstloc_in, dinv_sb, dinv2_sb, dsqrt_sb, ones_col,
                )

    nc.compile()
    return nc



def _emit_combine_phase(nc, mybir, work, p, u2s, u3s, saves, dsqrt_sb, out):
    """out_i for windows [7p, 7p+7): theta-mix u0..u3, scale by dsqrt, store.
    Runs inside hop 3's phase loop so it overlaps the remaining gathers."""
    f32 = mybir.dt.float32
    Alu = mybir.AluOpType
    w0 = p * G_WIN
    cs = slice(w0 * H, (w0 + G_WIN) * H)
    u0c = work.tile([128, G_WIN * H], f32, tag="u0c", bufs=2)
    nc.sync.dma_start(out=u0c[:], in_=saves[0][:, cs])
    u1c = work.tile([128, G_WIN * H], f32, tag="u1c", bufs=2)
    nc.sync.dma_start(out=u1c[:], in_=saves[1][:, cs])
    us = [u0c[:], u1c[:], u2s[:, cs], u3s[:, cs]]
    for i in range(4):
        acc = work.tile([128, G_WIN * H], f32, tag="acc", bufs=2)
        ks = [kk for kk in range(4) if THETAS[i][kk] != 0.0]
        nc.vector.tensor_scalar(
            out=acc[:], in0=us[ks[0]],
            scalar1=float(THETAS[i][ks[0]]), scalar2=None, op0=Alu.mult,
        )
        for kk in ks[1:]:
            tmp = work.tile([128, G_WIN * H], f32, tag="ctmp", bufs=2)
            nc.vector.tensor_scalar(
                out=tmp[:], in0=us[kk],
                scalar1=float(THETAS[i][kk]), scalar2=None, op0=Alu.mult,
            )
            nc.vector.tensor_tensor(out=acc[:], in0=acc[:], in1=tmp[:], op=Alu.add)
        nc.vector.tensor_tensor(
            out=acc[:].rearrange("q (w h) -> q w h", h=H),
            in0=acc[:].rearrange("q (w h) -> q w h", h=H),
            in1=dsqrt_sb[:, w0 : w0 + G_WIN].to_broadcast([128, G_WIN, H]),
            op=Alu.mult,
        )
        nc.sync.dma_start(
            out=out[w0 * 128 : (w0 + G_WIN) * 128, i * H : (i + 1) * H].rearrange(
                "(w q) h -> q w h", q=128
            ),
            in_=acc[:].rearrange("q (w h) -> q w h", h=H),
        )


def _emit_body(
    nc, tc, mybir, rep, T, tot_tiles, phase_tiles, call_info, tile_start,
    dram, work, msgs_pool, psum_pool, slab_a, slab_b, saves, ag_in, feat_in,
    out, w_sb, b_sb, ident, iota_sb, idx_in, dstloc_in, dinv_sb, dinv2_sb,
    dsqrt_sb, ones_col,
):
    import concourse.mybir as mybir

    f32 = mybir.dt.float32
    i16 = mybir.dt.int16
    Alu = mybir.AluOpType
    SLAB = WPC * H
    max_ptiles = max(phase_tiles)

    SKIP_COMPUTE = bool(int(os.environ.get("BW_SKIP_COMPUTE", "0")))
    SKIP_GATHER = bool(int(os.environ.get("BW_SKIP_GATHER", "0")))
    if True:  # keep indentation of original body
            # ---------- u0 = Dinv * leaky_relu(feat @ W + b)
            with tc.tile_pool(name=f"featp{rep}", bufs=2) as featp:
                for w0 in range(0, WPC, G_WIN):
                  gw = min(G_WIN, WPC - w0)
                  feat_sb = featp.tile([128, G_WIN, F_IN], f32, tag="feat_sb")
                  nc.sync.dma_start(
                      out=feat_sb[:, :gw, :],
                      in_=feat_in[w0 * 128 : (w0 + gw) * 128, :].rearrange(
                          "(w p) f -> p w f", p=128
                      ),
                  )
                  for w in range(w0, w0 + gw):
                    ftT_ps = psum_pool.tile([128, 128], f32, tag="ps", bufs=8)
                    nc.tensor.transpose(
                        out=ftT_ps[:], in_=feat_sb[:, w - w0, :], identity=ident[:]
                    )
                    ftT = work.tile([128, 128], f32, tag="ftT_sb")
                    nc.vector.tensor_copy(out=ftT[:], in_=ftT_ps[:])
                    h_ps = psum_pool.tile([128, 128], f32, tag="ps", bufs=8)
                    nc.tensor.matmul(
                        out=h_ps[:, :H], lhsT=ftT[:], rhs=w_sb[:],
                        start=True, stop=False,
                    )
                    nc.tensor.matmul(
                        out=h_ps[:, :H], lhsT=ones_col[:], rhs=b_sb[:],
                        start=False, stop=True,
                    )
                    t1 = work.tile([128, H], f32, tag="t1")
                    nc.vector.tensor_scalar(
                        out=t1[:], in0=h_ps[:, :H], scalar1=0.01, scalar2=None,
                        op0=Alu.mult,
                    )
                    t2 = work.tile([128, H], f32, tag="t2")
                    nc.vector.tensor_tensor(
                        out=t2[:], in0=h_ps[:, :H], in1=t1[:], op=Alu.max
                    )
                    nc.vector.tensor_scalar(
                        out=slab_a[:, w * H : (w + 1) * H], in0=t2[:],
                        scalar1=dinv_sb[:, w : w + 1], scalar2=None, op0=Alu.mult,
                    )

            def store_slab(slab, save_idx):
                if save_idx is not None:
                    nc.sync.dma_start(out=saves[save_idx][:], in_=slab[:])
                nc.sync.dma_start(
                    out=ag_in[:].rearrange("(w p) h -> p w h", p=128),
                    in_=slab[:].rearrange("p (w h) -> p w h", h=H),
                )

            store_slab(slab_a, 0)

            # ---------- hops
            cur, nxt = slab_a, slab_b
            for hop in range(NHOP):
                table = dram.tile(
                    [NTAB, H], f32, addr_space="Shared", name=f"table{rep}_{hop}"
                )
                nc.gpsimd.collective_compute(
                    "AllGather",
                    Alu.bypass,
                    replica_groups=[list(range(NCORES))],
                    ins=[ag_in.opt()],
                    outs=[table.opt()],
                )

                g = 0  # global tile counter
                for p in range(NPHASE):
                    ptiles = phase_tiles[p]
                    p_off16 = call_info[p][0][0] // 16  # phase idx col start
                    p_len16 = ptiles * 128 // 16
                    first_g_p = call_info[p][0][0] // 128
                    idxp = msgs_pool.tile(
                        [128, (max_ptiles * 128) // 16], i16, tag="idxp"
                    )
                    nc.sync.dma_start(
                        out=idxp[:, :p_len16],
                        in_=idx_in[:, p_off16 : p_off16 + p_len16],
                    )
                    dstp = msgs_pool.tile(
                        [128, max_ptiles + SBATCH], f32, tag="dstp"
                    )
                    nc.sync.dma_start(
                        out=dstp[:, : ptiles + SBATCH],
                        in_=dstloc_in[:, first_g_p : first_g_p + ptiles + SBATCH],
                    )
                    msgs = msgs_pool.tile([128, max_ptiles, H], f32, tag="msgs")
                    if SKIP_GATHER and (hop > 0 or p > 1):
                        nc.vector.memset(msgs[:, 0, :], 0.5)
                    GCAP = 8192  # max idxs per dma_gather (desc-ring capacity)
                    for kk in range(NCHUNK):
                        off_sl, n_sl, col = call_info[p][kk]
                        if SKIP_GATHER and (hop > 0 or p > 1):
                            continue
                        for o in range(0, n_sl, GCAP):
                            ln = min(GCAP, n_sl - o)
                            c0 = col + o // 128
                            i0 = (off_sl + o) // 16 - p_off16
                            nc.gpsimd.dma_gather(
                                msgs[:, c0 : c0 + ln // 128, :],
                                table[CH_PAD * kk : CH_PAD * (kk + 1), :],
                                idxp[:, i0 : i0 + ln // 16],
                                ln,
                                ln,
                                H,
                                single_packet=False,
                                queue_num=kk % int(os.environ.get("BW_NSWDGE", "4")),
                            )
                    # S build for this phase's tiles
                    first_g = g
                    s_tiles = []
                    for sb0 in range(0, ptiles, SBATCH):
                        S_big = work.tile(
                            [128, SBATCH * 128], f32, tag="S", bufs=4
                        )
                        if SKIP_COMPUTE:
                            s_tiles.append(S_big)
                            continue
                        nc.vector.tensor_tensor(
                            out=S_big[:].rearrange("p (t j) -> p t j", j=128),
                            in0=iota_sb[:].rearrange("p (t j) -> p t j", j=128),
                            in1=dstp[:, sb0 : sb0 + SBATCH]
                            .to_broadcast([128, SBATCH, 128]),
                            op=Alu.is_equal,
                        )
                        s_tiles.append(S_big)
                    # matmuls per window
                    for ww in range(p * G_WIN, (p + 1) * G_WIN):
                        agg_ps = psum_pool.tile([128, 128], f32, tag="ps", bufs=8)
                        if SKIP_COMPUTE:
                            nc.vector.memset(agg_ps[:, :H], 0.0)
                        n_mm = int(sum(T[ww][kk] for kk in range(NCHUNK)))
                        mm_i = 0
                        for kk in range(NCHUNK if not SKIP_COMPUTE else 0):
                            _, _, col = call_info[p][kk]
                            # tiles of (ww,kk) within the call: windows before ww
                            cbase = col + int(
                                sum(T[w2][kk] for w2 in range(p * G_WIN, ww))
                            )
                            for t in range(int(T[ww][kk])):
                                # global tile index in host (p,k,w,t) order
                                lg = int(tile_start[ww][kk]) + t - first_g
                                S_t = s_tiles[lg // SBATCH]
                                j0 = (lg % SBATCH) * 128
                                nc.tensor.matmul(
                                    out=agg_ps[:, :H],
                                    lhsT=S_t[:, j0 : j0 + 128],
                                    rhs=msgs[:, cbase + t, :],
                                    start=(mm_i == 0),
                                    stop=(mm_i == n_mm - 1),
                                )
                                mm_i += 1
                        g += n_mm
                        # u' = u - dinv2 * agg
                        tscl = work.tile([128, H], f32, tag="tscl")
                        nc.vector.tensor_scalar(
                            out=tscl[:], in0=agg_ps[:, :H],
                            scalar1=dinv2_sb[:, ww : ww + 1], scalar2=None,
                            op0=Alu.mult,
                        )
                        nc.vector.tensor_tensor(
                            out=nxt[:, ww * H : (ww + 1) * H],
                            in0=cur[:, ww * H : (ww + 1) * H],
                            in1=tscl[:],
                            op=Alu.subtract,
                        )
                    if hop == NHOP - 1:
                        pass  # combine emitted per phase below
                    if hop == NHOP - 1 and ww == (p + 1) * G_WIN - 1:
                        _emit_combine_phase(
                            nc, mybir, work, p, cur, nxt, saves, dsqrt_sb, out
                        )
                assert g == sum(phase_tiles[:NPHASE])
                if hop == 0:
                    store_slab(nxt, 1)  # save u1
                elif hop < NHOP - 1:
                    store_slab(nxt, None)  # u2 stays in SBUF (slab_a)
                cur, nxt = nxt, cur



# ---------------------------------------------------------------- runner
def _make_runner(nc, in_maps, n_cores):
    import jax
    import numpy as np
    from jax.sharding import Mesh, NamedSharding, PartitionSpec
    from jax.experimental.shard_map import shard_map

    import concourse.mybir as mybir
    from concourse import bass2jax

    bass2jax.install_neuronx_cc_hook()
    partition_name = nc.partition_id_tensor.name if nc.partition_id_tensor else None
    in_names, out_names, out_avals, zero_outs = [], [], [], []
    for alloc in nc.m.functions[0].allocations:
        if not isinstance(alloc, mybir.MemoryLocationSet):
            continue
        name = alloc.memorylocations[0].name
        if alloc.kind == "ExternalInput":
            if name != partition_name:
                in_names.append(name)
        elif alloc.kind == "ExternalOutput":
            out_names.append(name)
            shape = tuple(alloc.tensor_shape)
            dtype = mybir.dt.np(alloc.dtype)
            out_avals.append(jax.core.ShapedArray(shape, dtype))
            zero_outs.append(np.zeros(shape, dtype))
    n_params = len(in_names)
    all_in_names = list(in_names) + list(out_names)
    if partition_name is not None:
        all_in_names.append(partition_name)

    def _body(*args):
        operands = list(args)
        if partition_name is not None:
            operands.append(bass2jax.partition_id_tensor())
        outs = bass2jax._bass_exec_p.bind(
            *operands,
            out_avals=tuple(out_avals),
            in_names=tuple(all_in_names),
            out_names=tuple(out_names),
            lowering_input_output_aliases=(),
            sim_require_finite=True,
            sim_require_nnan=True,
            nc=nc,
        )
        return tuple(outs)

    devices = jax.devices()[:n_cores]
    mesh = Mesh(np.asarray(devices), ("core",))
    n_ops = n_params + len(out_names)
    sharded = jax.jit(
        shard_map(
            _body,
            mesh=mesh,
            in_specs=(PartitionSpec("core"),) * n_ops,
            out_specs=(PartitionSpec("core"),) * len(out_names),
            check_rep=False,
        ),
        keep_unused=True,
    )
    sh = NamedSharding(mesh, PartitionSpec("core"))
    concat_in = [
        jax.device_put(
            np.concatenate([np.asarray(in_maps[c][nm]) for c in range(n_cores)], 0),
            sh,
        )
        for nm in in_names
    ]
    concat_zeros = [
        jax.device_put(np.zeros((n_cores * z.shape[0], *z.shape[1:]), z.dtype), sh)
        for z in zero_outs
    ]
    args = concat_in + concat_zeros

    def run():
        return sharded(*args)

    return run, out_names, out_avals


_CACHE = {}


def _get_built(edge_index_bytes_key, edge_index):
    if edge_index_bytes_key not in _CACHE:
        prep = _prep(edge_index)
        nc = _build_nc(
            prep["T"],
            prep["tot_tiles"],
            prep["phase_tiles"],
            prep["call_info"],
            prep["tile_start"],
        )
        _CACHE[edge_index_bytes_key] = (prep, nc)
    return _CACHE[edge_index_bytes_key]


def _make_in_maps(prep, inputs):
    feature = np.asarray(inputs["feature"], dtype=np.float32)
    W = np.asarray(inputs["W"], dtype=np.float32)
    b = np.asarray(inputs["b"], dtype=np.float32)

    iota = np.broadcast_to(
        np.tile(np.arange(128, dtype=np.float32), SBATCH), (128, SBATCH * 128)
    ).copy()
    ident = np.eye(128, dtype=np.float32)
    b2 = b.reshape(1, H)

    feat_pad = np.zeros((NCORES, NPC_PAD, F_IN), dtype=np.float32)
    feat_pad[:, :NPC, :] = feature.reshape(NCORES, NPC, F_IN)

    in_maps = []
    for c in range(NCORES):
        idx_w, dl_w = prep["per_core"][c]
        in_maps.append(
            {
                "feat_in": feat_pad[c],
                "w_in": W,
                "b_in": b2,
                "ident_in": ident,
                "iota_in": iota,
                "idx_in": idx_w,
                "dstloc_in": dl_w,
                "dinv_in": prep["dinv_t"][c],
                "dinv2_in": prep["dinv2_t"][c],
                "dsqrt_in": prep["dsqrt_t"][c],
            }
        )
    return in_maps


def kernel(feature, edge_index, W, b):
    import jax

    edge_index = np.asarray(edge_index, dtype=np.int32)
    key = hash(edge_index.tobytes())
    prep, nc = _get_built(key, edge_index)
    in_maps = _make_in_maps(prep, {"feature": feature, "W": W, "b": b})

    run, out_names, out_avals = _make_runner(nc, in_maps, NCORES)
    outs = jax.block_until_ready(run())
    oi = out_names.index("out")
    full = np.asarray(outs[oi]).reshape(NCORES, NPC_PAD, 4 * H)
    return full[:, :NPC, :].reshape(N, 4 * H)


if __name__ == "__main__":
    rng = np.random.default_rng(0)
    feature = rng.standard_normal((N, F_IN), dtype=np.float32)
    edge_index = rng.integers(0, N, (2, E)).astype(np.int32)
    W = (rng.standard_normal((F_IN, H)) * 0.05).astype(np.float32)
    b = (rng.standard_normal((H,)) * 0.05).astype(np.float32)
    out = kernel(feature=feature, edge_index=edge_index, W=W, b=b)
    print(out.shape, out.dtype, float(np.abs(out).mean()))



# revision 23
# speedup vs baseline: 1.4837x; 1.0462x over previous
"""BWGNN (Beta Wavelet GNN) Trainium2 kernel, 8-way SPMD.

Math (reference.py): deg = out-degree(src) clamped >=1; Dinv = deg^-1/2;
h = leaky_relu(feature @ W + b); L feat = feat - Dinv*segsum_dst(Dinv[src]*feat[src]);
out = concat_i sum_k THETA[i][k] L^k h.

We iterate on u_k = Dinv * L^k h:
    u_{k+1} = u_k - Dinv^2 * segsum_dst(u_k[src])
    out_i   = (sum_k THETA[i][k] u_k) * deg^{1/2}

Distribution: nodes dst-sharded over 8 cores (12500 + pad -> 12544 rows/core).
Full u-table [8*12544, 64] lives in each core's HBM, refreshed per hop by an
AllGather of the per-core updated slices (same-chip AG is cheap).

Per hop per core: edges owned by the core (dst in slice) are bucketed by
(dst-window of 128 nodes, src-chunk of 25088 padded rows), each bucket padded
to a multiple of 128 edges. dma_gather (SWDGE, int16 local idx) pulls
u[src] rows from the HBM table into SBUF edge-tiles; a one-hot matrix
S[p, j] = (dst_loc[p] == j) built on DVE turns the per-window segment-sum
into PE matmuls accumulating in PSUM; the window's PSUM drains through the
u-update (2 DVE ops) into the next-u SBUF slab.
"""

import math
import os
import sys

sys.path.insert(0, "/opt/trn_rl_repo")

import numpy as np

# ---------------------------------------------------------------- constants
N = 100000
E = 1600000
F_IN = 128
H = 64
NCORES = 8
NPC = 12500          # nodes per core
WPC = 98             # windows (128-node groups) per core
NPC_PAD = WPC * 128  # 12544
NCHUNK = 4
CH_NODES = 25000     # original nodes per chunk
CH_PAD = 2 * NPC_PAD  # 25088 padded rows per chunk
NTAB = NCORES * NPC_PAD  # 100352
G_WIN = 7            # windows per phase
NPHASE = WPC // G_WIN  # 14
SBATCH = 8           # S matrices built per DVE instruction
NHOP = 3


def _calculate_theta2(d):
    thetas = []
    for i in range(d):
        c1 = np.zeros(i + 1)
        c1[i] = 0.5 ** i
        c2 = np.array([math.comb(d - i, j) * (-0.5) ** j for j in range(d - i + 1)])
        c = np.convolve(c1, c2)
        B = math.factorial(i) * math.factorial(d - i) / math.factorial(d + 1)
        c = c / (2.0 * B)
        thetas.append([float(c[j]) for j in range(d)])
    return thetas


THETAS = _calculate_theta2(4)  # [4][4], theta[i][k] weight of L^k h in output i


# ---------------------------------------------------------------- host prep
def _prep(edge_index: np.ndarray):
    """Bucket edges, build per-core gather-index / dst-loc arrays and the
    static tile-count table T[w][k] (shared by all cores)."""
    src = edge_index[0].astype(np.int64)
    dst = edge_index[1].astype(np.int64)

    deg = np.bincount(src, minlength=N).astype(np.float32)
    dinv = np.maximum(deg, np.float32(1.0)) ** np.float32(-0.5)
    dinv2 = dinv * dinv
    dsqrt = np.float32(1.0) / dinv  # = max(deg,1)^0.5

    core = dst // NPC
    w = (dst % NPC) // 128
    dst_loc = (dst % NPC) % 128
    k = src // CH_NODES
    # padded row within chunk
    src_loc = (src % NPC) + (src // NPC - 2 * k) * NPC_PAD

    bucket = ((core * WPC + w) * NCHUNK + k)
    cnt = np.bincount(bucket, minlength=NCORES * WPC * NCHUNK).reshape(
        NCORES, WPC, NCHUNK
    )
    T = np.maximum(1, -(-cnt // 128)).max(axis=0)  # [WPC, NCHUNK] int64
    ncols = T * 1  # tiles per (w,k)

    # sort edges by bucket (stable, any order within bucket).  In
    # BW_IDX_MODE=sort, additionally order each bucket's edges by source row
    # so consecutive gather descriptors hit nearby HBM addresses (segment-sum
    # is order-invariant, so results are unchanged).
    idx_mode = os.environ.get("BW_IDX_MODE", "")
    if idx_mode == "sort":
        order = np.lexsort((src_loc, bucket))
    else:
        order = np.argsort(bucket, kind="stable")
    src_loc_s = src_loc[order]
    dst_loc_s = dst_loc[order]
    bucket_s = bucket[order]
    # start offset of each (c,w,k) in the sorted arrays
    starts = np.zeros(NCORES * WPC * NCHUNK + 1, dtype=np.int64)
    np.cumsum(np.bincount(bucket_s, minlength=NCORES * WPC * NCHUNK), out=starts[1:])

    # global slot enumeration per core: for p, k, w in phase, t, slot
    # (tile g covers slots [128g, 128(g+1)))
    tot_tiles = int(T.sum())
    tot_slots = tot_tiles * 128

    # per-(p,k): column base within phase msgs tile and call length
    # call order within phase: k = 0..3
    phase_tiles = []  # [p] -> total tiles in phase
    call_info = []  # [p][k] = (idx_off_slots, n_slots, col_base)
    gcol = 0  # running global tile index
    for p in range(NPHASE):
        ws = range(p * G_WIN, (p + 1) * G_WIN)
        info = []
        col = 0
        for kk in range(NCHUNK):
            n_t = int(sum(T[ww][kk] for ww in ws))
            info.append((gcol * 128, n_t * 128, col))
            col += n_t
            gcol += n_t
        call_info.append(info)
        phase_tiles.append(col)
    assert gcol == tot_tiles

    # map (w,k) -> global tile start
    tile_start = np.zeros((WPC, NCHUNK), dtype=np.int64)
    g = 0
    for p in range(NPHASE):
        for kk in range(NCHUNK):
            for ww in range(p * G_WIN, (p + 1) * G_WIN):
                tile_start[ww][kk] = g
                g += T[ww][kk]

    # build per-core slot arrays
    per_core = []
    for c in range(NCORES):
        idx_arr = np.zeros(tot_slots, dtype=np.int16)
        dl_arr = np.full(tot_slots, -1.0, dtype=np.float32)
        for ww in range(WPC):
            for kk in range(NCHUNK):
                b = (c * WPC + ww) * NCHUNK + kk
                s0, s1 = starts[b], starts[b + 1]
                n = s1 - s0
                o = tile_start[ww][kk] * 128
                cap = T[ww][kk] * 128
                assert n <= cap
                idx_arr[o : o + n] = src_loc_s[s0:s1]
                dl_arr[o : o + n] = dst_loc_s[s0:s1]
                if n < cap:  # pad with valid idxs, dst -1; spread the pad
                    # rows across the quarter table to avoid HBM bank
                    # conflicts (all-pads-on-row-0 measured 67% slower)
                    npad = cap - n
                    if os.environ.get("BW_PAD_MODE", "spread") == "spread":
                        idx_arr[o + n : o + cap] = (
                            (o + n + np.arange(npad)) * 509
                        ) % CH_PAD
                    else:
                        idx_arr[o + n : o + cap] = 0
        if idx_mode == "zero":  # timing probe: every gather hits row 0
            idx_arr[:] = 0
        # wrap idx: position i -> [16r + i%16, i//16]
        idx_w = idx_arr.reshape(-1, 16).T  # [16, tot/16]
        idx_w = np.tile(idx_w, (8, 1))  # [128, tot/16]
        # dst_loc: tile g slot s -> [s, g]
        dl_w = dl_arr.reshape(tot_tiles, 128).T.copy()  # [128, tot_tiles]
        # pad dstloc columns for S-batch overrun
        dl_w = np.concatenate(
            [dl_w, np.full((128, SBATCH), -1.0, dtype=np.float32)], axis=1
        )
        per_core.append((idx_w, dl_w))

    # per-core dinv arrays [128, WPC] (pad nodes -> 1.0)
    def slice_arr(a):
        out = np.ones((NCORES, NPC_PAD), dtype=np.float32)
        out[:, :NPC] = a.reshape(NCORES, NPC)
        return out.reshape(NCORES, WPC, 128).transpose(0, 2, 1).copy()

    return {
        "T": T,
        "tot_tiles": tot_tiles,
        "phase_tiles": phase_tiles,
        "call_info": call_info,
        "tile_start": tile_start,
        "per_core": per_core,
        "dinv_t": slice_arr(dinv),
        "dinv2_t": slice_arr(dinv2),
        "dsqrt_t": slice_arr(dsqrt),
    }


# ---------------------------------------------------------------- bass build
def _build_nc(T, tot_tiles, phase_tiles, call_info, tile_start, reps=1):
    import concourse.bacc as bacc
    import concourse.mybir as mybir
    import concourse.tile as tile
    from concourse.library_config import mlp

    f32 = mybir.dt.float32
    i16 = mybir.dt.int16
    Alu = mybir.AluOpType

    tot16 = tot_tiles * 128 // 16
    dl_cols = tot_tiles + SBATCH
    max_ptiles = max(phase_tiles)
    nsb = -(-tot_tiles // SBATCH)  # number of S-batches overall

    nq = int(os.environ.get("BW_NSWDGE", "4"))
    nc = bacc.Bacc("TRN2", target_bir_lowering=False, debug=False,
                   num_devices=NCORES, num_swdge_queues=nq)

    feat_in = nc.dram_tensor("feat_in", [NPC_PAD, F_IN], f32, kind="ExternalInput")
    w_in = nc.dram_tensor("w_in", [F_IN, H], f32, kind="ExternalInput")
    b_in = nc.dram_tensor("b_in", [1, H], f32, kind="ExternalInput")
    ident_in = nc.dram_tensor("ident_in", [128, 128], f32, kind="ExternalInput")
    iota_in = nc.dram_tensor("iota_in", [128, SBATCH * 128], f32, kind="ExternalInput")
    idx_in = nc.dram_tensor("idx_in", [128, tot16], i16, kind="ExternalInput")
    dstloc_in = nc.dram_tensor("dstloc_in", [128, dl_cols], f32, kind="ExternalInput")
    dinv_in = nc.dram_tensor("dinv_in", [128, WPC], f32, kind="ExternalInput")
    dinv2_in = nc.dram_tensor("dinv2_in", [128, WPC], f32, kind="ExternalInput")
    dsqrt_in = nc.dram_tensor("dsqrt_in", [128, WPC], f32, kind="ExternalInput")
    out = nc.dram_tensor("out", [NPC_PAD, 4 * H], f32, kind="ExternalOutput")

    SLAB = WPC * H  # 6272 free elems

    with tile.TileContext(nc) as tc:
        with (
            tc.tile_pool(name="dram", bufs=1, space="DRAM") as dram,
            tc.tile_pool(name="const", bufs=1) as const,
            tc.tile_pool(name="slabs", bufs=1) as slabs,
            tc.tile_pool(name="work", bufs=3) as work,
            tc.tile_pool(name="msgs_p", bufs=2) as msgs_pool,
            tc.tile_pool(name="psum", bufs=4, space="PSUM") as psum_pool,
        ):
            nc.gpsimd.load_library(mlp)

            # ---------- constants / metadata loads
            w_sb = const.tile([F_IN, H], f32)
            nc.sync.dma_start(out=w_sb[:], in_=w_in[:])
            b_sb = const.tile([1, H], f32)
            nc.sync.dma_start(out=b_sb[:], in_=b_in[:])
            ident = const.tile([128, 128], f32)
            nc.sync.dma_start(out=ident[:], in_=ident_in[:])
            iota_sb = const.tile([128, SBATCH * 128], f32)
            nc.sync.dma_start(out=iota_sb[:], in_=iota_in[:])
            dinv_sb = const.tile([128, WPC], f32)
            nc.sync.dma_start(out=dinv_sb[:], in_=dinv_in[:])
            dinv2_sb = const.tile([128, WPC], f32)
            nc.sync.dma_start(out=dinv2_sb[:], in_=dinv2_in[:])
            dsqrt_sb = const.tile([128, WPC], f32)
            nc.sync.dma_start(out=dsqrt_sb[:], in_=dsqrt_in[:])
            ones_col = const.tile([1, 128], f32)
            nc.vector.memset(ones_col[:], 1.0)

            slab_a = slabs.tile([128, SLAB], f32)  # u0 -> u2
            slab_b = slabs.tile([128, SLAB], f32)  # u1 -> u3

            saves = [
                dram.tile([128, SLAB], f32, name=f"save{kk}") for kk in range(2)
            ]
            ag_bufs = [
                dram.tile([NPC_PAD, H], f32, name=f"agb{i}") for i in range(2)
            ]

            for rep in range(reps):
                _emit_body(
                    nc, tc, mybir, rep, T, tot_tiles, phase_tiles, call_info,
                    tile_start, dram, work, msgs_pool, psum_pool, slab_a, slab_b,
                    saves, ag_bufs, feat_in, out, w_sb, b_sb, ident, iota_sb,
                    idx_in, dstloc_in, dinv_sb, dinv2_sb, dsqrt_sb, ones_col,
                )

    nc.compile()
    return nc



def _emit_combine_phase(nc, mybir, work, p, u2s, u3s, saves, dsqrt_sb, out):
    """out_i for windows [7p, 7p+7): theta-mix u0..u3, scale by dsqrt, store.
    Runs inside hop 3's phase loop so it overlaps the remaining gathers."""
    f32 = mybir.dt.float32
    Alu = mybir.AluOpType
    w0 = p * G_WIN
    cs = slice(w0 * H, (w0 + G_WIN) * H)
    u0c = work.tile([128, G_WIN * H], f32, tag="u0c", bufs=2)
    nc.sync.dma_start(out=u0c[:], in_=saves[0][:, cs])
    u1c = work.tile([128, G_WIN * H], f32, tag="u1c", bufs=2)
    nc.sync.dma_start(out=u1c[:], in_=saves[1][:, cs])
    us = [u0c[:], u1c[:], u2s[:, cs], u3s[:, cs]]
    for i in range(4):
        acc = work.tile([128, G_WIN * H], f32, tag="acc", bufs=2)
        ks = [kk for kk in range(4) if THETAS[i][kk] != 0.0]
        nc.vector.tensor_scalar(
            out=acc[:], in0=us[ks[0]],
            scalar1=float(THETAS[i][ks[0]]), scalar2=None, op0=Alu.mult,
        )
        for kk in ks[1:]:
            tmp = work.tile([128, G_WIN * H], f32, tag="ctmp", bufs=2)
            nc.vector.tensor_scalar(
                out=tmp[:], in0=us[kk],
                scalar1=float(THETAS[i][kk]), scalar2=None, op0=Alu.mult,
            )
            nc.vector.tensor_tensor(out=acc[:], in0=acc[:], in1=tmp[:], op=Alu.add)
        nc.vector.tensor_tensor(
            out=acc[:].rearrange("q (w h) -> q w h", h=H),
            in0=acc[:].rearrange("q (w h) -> q w h", h=H),
            in1=dsqrt_sb[:, w0 : w0 + G_WIN].to_broadcast([128, G_WIN, H]),
            op=Alu.mult,
        )
        nc.sync.dma_start(
            out=out[w0 * 128 : (w0 + G_WIN) * 128, i * H : (i + 1) * H].rearrange(
                "(w q) h -> q w h", q=128
            ),
            in_=acc[:].rearrange("q (w h) -> q w h", h=H),
        )


def _emit_body(
    nc, tc, mybir, rep, T, tot_tiles, phase_tiles, call_info, tile_start,
    dram, work, msgs_pool, psum_pool, slab_a, slab_b, saves, ag_bufs, feat_in,
    out, w_sb, b_sb, ident, iota_sb, idx_in, dstloc_in, dinv_sb, dinv2_sb,
    dsqrt_sb, ones_col,
):
    import concourse.mybir as mybir

    f32 = mybir.dt.float32
    i16 = mybir.dt.int16
    Alu = mybir.AluOpType
    SLAB = WPC * H
    max_ptiles = max(phase_tiles)

    SKIP_COMPUTE = bool(int(os.environ.get("BW_SKIP_COMPUTE", "0")))
    SKIP_GATHER = bool(int(os.environ.get("BW_SKIP_GATHER", "0")))
    ag_in = ag_bufs[0]
    if True:  # keep indentation of original body
            # ---------- u0 = Dinv * leaky_relu(feat @ W + b)
            with tc.tile_pool(name=f"featp{rep}", bufs=2) as featp:
                for w0 in range(0, WPC, G_WIN):
                  gw = min(G_WIN, WPC - w0)
                  feat_sb = featp.tile([128, G_WIN, F_IN], f32, tag="feat_sb")
                  nc.sync.dma_start(
                      out=feat_sb[:, :gw, :],
                      in_=feat_in[w0 * 128 : (w0 + gw) * 128, :].rearrange(
                          "(w p) f -> p w f", p=128
                      ),
                  )
                  for w in range(w0, w0 + gw):
                    ftT_ps = psum_pool.tile([128, 128], f32, tag="ps", bufs=8)
                    nc.tensor.transpose(
                        out=ftT_ps[:], in_=feat_sb[:, w - w0, :], identity=ident[:]
                    )
                    ftT = work.tile([128, 128], f32, tag="ftT_sb")
                    nc.vector.tensor_copy(out=ftT[:], in_=ftT_ps[:])
                    h_ps = psum_pool.tile([128, 128], f32, tag="ps", bufs=8)
                    nc.tensor.matmul(
                        out=h_ps[:, :H], lhsT=ftT[:], rhs=w_sb[:],
                        start=True, stop=False,
                    )
                    nc.tensor.matmul(
                        out=h_ps[:, :H], lhsT=ones_col[:], rhs=b_sb[:],
                        start=False, stop=True,
                    )
                    t1 = work.tile([128, H], f32, tag="t1")
                    nc.vector.tensor_scalar(
                        out=t1[:], in0=h_ps[:, :H], scalar1=0.01, scalar2=None,
                        op0=Alu.mult,
                    )
                    t2 = work.tile([128, H], f32, tag="t2")
                    nc.vector.tensor_tensor(
                        out=t2[:], in0=h_ps[:, :H], in1=t1[:], op=Alu.max
                    )
                    nc.vector.tensor_scalar(
                        out=slab_a[:, w * H : (w + 1) * H], in0=t2[:],
                        scalar1=dinv_sb[:, w : w + 1], scalar2=None, op0=Alu.mult,
                    )

            def store_slab(slab, save_idx):
                if save_idx is not None:
                    nc.sync.dma_start(out=saves[save_idx][:], in_=slab[:])
                nc.sync.dma_start(
                    out=ag_in[:].rearrange("(w p) h -> p w h", p=128),
                    in_=slab[:].rearrange("p (w h) -> p w h", h=H),
                )

            store_slab(slab_a, 0)

            # ---------- hops
            cur, nxt = slab_a, slab_b
            for hop in range(NHOP):
                table = dram.tile(
                    [NTAB, H], f32, addr_space="Shared", name=f"table{rep}_{hop}"
                )
                nc.gpsimd.collective_compute(
                    "AllGather",
                    Alu.bypass,
                    replica_groups=[list(range(NCORES))],
                    ins=[ag_in.opt()],
                    outs=[table.opt()],
                )

                g = 0  # global tile counter
                for p in range(NPHASE):
                    ptiles = phase_tiles[p]
                    p_off16 = call_info[p][0][0] // 16  # phase idx col start
                    p_len16 = ptiles * 128 // 16
                    first_g_p = call_info[p][0][0] // 128
                    idxp = msgs_pool.tile(
                        [128, (max_ptiles * 128) // 16], i16, tag="idxp"
                    )
                    nc.sync.dma_start(
                        out=idxp[:, :p_len16],
                        in_=idx_in[:, p_off16 : p_off16 + p_len16],
                    )
                    dstp = msgs_pool.tile(
                        [128, max_ptiles + SBATCH], f32, tag="dstp"
                    )
                    nc.sync.dma_start(
                        out=dstp[:, : ptiles + SBATCH],
                        in_=dstloc_in[:, first_g_p : first_g_p + ptiles + SBATCH],
                    )
                    msgs = msgs_pool.tile([128, max_ptiles, H], f32, tag="msgs")
                    if SKIP_GATHER and (hop > 0 or p > 1):
                        nc.vector.memset(msgs[:, 0, :], 0.5)
                    GCAP = 8192  # max idxs per dma_gather (desc-ring capacity)
                    for kk in range(NCHUNK):
                        off_sl, n_sl, col = call_info[p][kk]
                        if SKIP_GATHER and (hop > 0 or p > 1):
                            continue
                        for o in range(0, n_sl, GCAP):
                            ln = min(GCAP, n_sl - o)
                            c0 = col + o // 128
                            i0 = (off_sl + o) // 16 - p_off16
                            nc.gpsimd.dma_gather(
                                msgs[:, c0 : c0 + ln // 128, :],
                                table[CH_PAD * kk : CH_PAD * (kk + 1), :],
                                idxp[:, i0 : i0 + ln // 16],
                                ln,
                                ln,
                                H,
                                single_packet=False,
                                queue_num=kk % int(os.environ.get("BW_NSWDGE", "4")),
                            )
                    # S build for this phase's tiles
                    first_g = g
                    s_tiles = []
                    for sb0 in range(0, ptiles, SBATCH):
                        S_big = work.tile(
                            [128, SBATCH * 128], f32, tag="S", bufs=4
                        )
                        if SKIP_COMPUTE:
                            s_tiles.append(S_big)
                            continue
                        nc.vector.tensor_tensor(
                            out=S_big[:].rearrange("p (t j) -> p t j", j=128),
                            in0=iota_sb[:].rearrange("p (t j) -> p t j", j=128),
                            in1=dstp[:, sb0 : sb0 + SBATCH]
                            .to_broadcast([128, SBATCH, 128]),
                            op=Alu.is_equal,
                        )
                        s_tiles.append(S_big)
                    # matmuls per window
                    for ww in range(p * G_WIN, (p + 1) * G_WIN):
                        agg_ps = psum_pool.tile([128, 128], f32, tag="ps", bufs=8)
                        if SKIP_COMPUTE:
                            nc.vector.memset(agg_ps[:, :H], 0.0)
                        n_mm = int(sum(T[ww][kk] for kk in range(NCHUNK)))
                        mm_i = 0
                        for kk in range(NCHUNK if not SKIP_COMPUTE else 0):
                            _, _, col = call_info[p][kk]
                            # tiles of (ww,kk) within the call: windows before ww
                            cbase = col + int(
                                sum(T[w2][kk] for w2 in range(p * G_WIN, ww))
                            )
                            for t in range(int(T[ww][kk])):
                                # global tile index in host (p,k,w,t) order
                                lg = int(tile_start[ww][kk]) + t - first_g
                                S_t = s_tiles[lg // SBATCH]
                                j0 = (lg % SBATCH) * 128
                                nc.tensor.matmul(
                                    out=agg_ps[:, :H],
                                    lhsT=S_t[:, j0 : j0 + 128],
                                    rhs=msgs[:, cbase + t, :],
                                    start=(mm_i == 0),
                                    stop=(mm_i == n_mm - 1),
                                )
                                mm_i += 1
                        g += n_mm
                        # u' = u - dinv2 * agg
                        tscl = work.tile([128, H], f32, tag="tscl")
                        nc.vector.tensor_scalar(
                            out=tscl[:], in0=agg_ps[:, :H],
                            scalar1=dinv2_sb[:, ww : ww + 1], scalar2=None,
                            op0=Alu.mult,
                        )
                        nc.vector.tensor_tensor(
                            out=nxt[:, ww * H : (ww + 1) * H],
                            in0=cur[:, ww * H : (ww + 1) * H],
                            in1=tscl[:],
                            op=Alu.subtract,
                        )
                    if hop == NHOP - 1:
                        pass  # combine emitted per phase below
                    if hop == NHOP - 1 and ww == (p + 1) * G_WIN - 1:
                        _emit_combine_phase(
                            nc, mybir, work, p, cur, nxt, saves, dsqrt_sb, out
                        )
                assert g == sum(phase_tiles[:NPHASE])
                if hop == 0:
                    store_slab(nxt, 1)  # save u1
                elif hop < NHOP - 1:
                    store_slab(nxt, None)  # u2 stays in SBUF (slab_a)
                cur, nxt = nxt, cur



# ---------------------------------------------------------------- runner
def _make_runner(nc, in_maps, n_cores):
    import jax
    import numpy as np
    from jax.sharding import Mesh, NamedSharding, PartitionSpec
    from jax.experimental.shard_map import shard_map

    import concourse.mybir as mybir
    from concourse import bass2jax

    bass2jax.install_neuronx_cc_hook()
    partition_name = nc.partition_id_tensor.name if nc.partition_id_tensor else None
    in_names, out_names, out_avals, zero_outs = [], [], [], []
    for alloc in nc.m.functions[0].allocations:
        if not isinstance(alloc, mybir.MemoryLocationSet):
            continue
        name = alloc.memorylocations[0].name
        if alloc.kind == "ExternalInput":
            if name != partition_name:
                in_names.append(name)
        elif alloc.kind == "ExternalOutput":
            out_names.append(name)
            shape = tuple(alloc.tensor_shape)
            dtype = mybir.dt.np(alloc.dtype)
            out_avals.append(jax.core.ShapedArray(shape, dtype))
            zero_outs.append(np.zeros(shape, dtype))
    n_params = len(in_names)
    all_in_names = list(in_names) + list(out_names)
    if partition_name is not None:
        all_in_names.append(partition_name)

    def _body(*args):
        operands = list(args)
        if partition_name is not None:
            operands.append(bass2jax.partition_id_tensor())
        outs = bass2jax._bass_exec_p.bind(
            *operands,
            out_avals=tuple(out_avals),
            in_names=tuple(all_in_names),
            out_names=tuple(out_names),
            lowering_input_output_aliases=(),
            sim_require_finite=True,
            sim_require_nnan=True,
            nc=nc,
        )
        return tuple(outs)

    devices = jax.devices()[:n_cores]
    mesh = Mesh(np.asarray(devices), ("core",))
    n_ops = n_params + len(out_names)
    sharded = jax.jit(
        shard_map(
            _body,
            mesh=mesh,
            in_specs=(PartitionSpec("core"),) * n_ops,
            out_specs=(PartitionSpec("core"),) * len(out_names),
            check_rep=False,
        ),
        keep_unused=True,
    )
    sh = NamedSharding(mesh, PartitionSpec("core"))
    concat_in = [
        jax.device_put(
            np.concatenate([np.asarray(in_maps[c][nm]) for c in range(n_cores)], 0),
            sh,
        )
        for nm in in_names
    ]
    concat_zeros = [
        jax.device_put(np.zeros((n_cores * z.shape[0], *z.shape[1:]), z.dtype), sh)
        for z in zero_outs
    ]
    args = concat_in + concat_zeros

    def run():
        return sharded(*args)

    return run, out_names, out_avals


_CACHE = {}


def _get_built(edge_index_bytes_key, edge_index):
    if edge_index_bytes_key not in _CACHE:
        prep = _prep(edge_index)
        nc = _build_nc(
            prep["T"],
            prep["tot_tiles"],
            prep["phase_tiles"],
            prep["call_info"],
            prep["tile_start"],
        )
        _CACHE[edge_index_bytes_key] = (prep, nc)
    return _CACHE[edge_index_bytes_key]


def _make_in_maps(prep, inputs):
    feature = np.asarray(inputs["feature"], dtype=np.float32)
    W = np.asarray(inputs["W"], dtype=np.float32)
    b = np.asarray(inputs["b"], dtype=np.float32)

    iota = np.broadcast_to(
        np.tile(np.arange(128, dtype=np.float32), SBATCH), (128, SBATCH * 128)
    ).copy()
    ident = np.eye(128, dtype=np.float32)
    b2 = b.reshape(1, H)

    feat_pad = np.zeros((NCORES, NPC_PAD, F_IN), dtype=np.float32)
    feat_pad[:, :NPC, :] = feature.reshape(NCORES, NPC, F_IN)

    in_maps = []
    for c in range(NCORES):
        idx_w, dl_w = prep["per_core"][c]
        in_maps.append(
            {
                "feat_in": feat_pad[c],
                "w_in": W,
                "b_in": b2,
                "ident_in": ident,
                "iota_in": iota,
                "idx_in": idx_w,
                "dstloc_in": dl_w,
                "dinv_in": prep["dinv_t"][c],
                "dinv2_in": prep["dinv2_t"][c],
                "dsqrt_in": prep["dsqrt_t"][c],
            }
        )
    return in_maps


def kernel(feature, edge_index, W, b):
    import jax

    edge_index = np.asarray(edge_index, dtype=np.int32)
    key = hash(edge_index.tobytes())
    prep, nc = _get_built(key, edge_index)
    in_maps = _make_in_maps(prep, {"feature": feature, "W": W, "b": b})

    run, out_names, out_avals = _make_runner(nc, in_maps, NCORES)
    outs = jax.block_until_ready(run())
    oi = out_names.index("out")
    full = np.asarray(outs[oi]).reshape(NCORES, NPC_PAD, 4 * H)
    return full[:, :NPC, :].reshape(N, 4 * H)


if __name__ == "__main__":
    rng = np.random.default_rng(0)
    feature = rng.standard_normal((N, F_IN), dtype=np.float32)
    edge_index = rng.integers(0, N, (2, E)).astype(np.int32)
    W = (rng.standard_normal((F_IN, H)) * 0.05).astype(np.float32)
    b = (rng.standard_normal((H,)) * 0.05).astype(np.float32)
    out = kernel(feature=feature, edge_index=edge_index, W=W, b=b)
    print(out.shape, out.dtype, float(np.abs(out).mean()))

